# revision 15
# baseline (speedup 1.0000x reference)
"""ATS (Adaptive Token Sampling) attention kernel for 8 Trainium2 NeuronCores.

Strategy
--------
Data-parallel over batch B=32 -> 4 samples per core.

The ATS index selection (cls-attention scores -> cdf -> argmin sampling ->
unique/dedup) has argmin tie margins down to 0 ulp, so it must match the
reference's jax-on-CPU arithmetic bitwise.  It is computed on the host with
the exact op sequence of the reference (sliced einsums that were verified
bitwise-equal to the reference's full einsum results).  Everything heavy
(K/V/Q projections, 257-row attention, AV, output projection: ~97 GFLOP)
runs on the NeuronCores.

Key algebraic simplification: the reference computes full [N,N] attention
then gathers S+1=257 rows.  Row-softmax commutes with row-gather, so the
device only computes attention for the 257 selected query rows (gathered on
the host as x_sel, since gather commutes with the linear qkv projection).

Device layout (per sample, all fp32):
  xT   [768,577] (host-pretransposed)   xselT [768,257]
  K_T  [768,577] = qkvT_k.T @ xT        (c' on partitions)
  V    [577,12*65] natural, head-strided with a ones column per head
  qsT  [768,257] = qkvT_q.T @ xselT
  per head h: dots_T[j,i] = K_T_h.T(slice) @ qsT_h   (j on partitions)
              exp_T = exp(dots_T/8)  (no max-subtraction: |dots|<~30 safe)
              o_aug[65,257] = V_aug_h.T @ exp_T  -> rows 0-63 numerator,
              row 64 = softmax denominator (from the ones column)
  normalize via reciprocal+broadcast-matmul, project with projT, bias added
  as a rank-1 matmul into the same PSUM accumulation, store outT [768,257].
"""

import sys

for _p in ("/opt/trn_rl_repo",):
    if _p not in sys.path:
        sys.path.insert(0, _p)

import numpy as np

B, N, C, H, HD, S = 32, 577, 768, 12, 64, 256
NCORES = 8
BL = B // NCORES            # samples per core
SP1 = S + 1                 # 257
SCALE = HD ** -0.5
EPS = 1e-06
JT = [128, 128, 128, 128, 65]   # j (token) tiles of N=577
CT = 6                          # 128-chunks of C=768
SP2 = SP1 + 1                   # 258: fp32r needs an even moving free dim
NJ = N + 1                      # 578: ditto for the K_T j dimension
USE_F32R = True                 # fp32r matmuls: 4x PE rate, slightly lower precision


# ----------------------------------------------------------------------------
# Host side: ATS index selection, bitwise-matching the jax-CPU reference.
# ----------------------------------------------------------------------------

def _host_selection(x, mask, qkv_w):
    import jax
    import jax.numpy as jnp

    cpu = jax.devices("cpu")[0]
    with jax.default_device(cpu):
        xj = jax.device_put(np.asarray(x), cpu)
        wj = jax.device_put(np.asarray(qkv_w), cpu)
        mj = jax.device_put(np.asarray(mask), cpu)

        # verified bitwise-equal to reference's full-qkv einsum slices
        kv = jnp.einsum('bnc,dc->bnd', xj, wj[C:])
        q0 = jnp.einsum('bc,dc->bd', xj[:, 0, :], wj[:C])
        k = kv[:, :, :C].reshape(B, N, H, HD).transpose(0, 2, 1, 3)
        v = kv[:, :, C:].reshape(B, N, H, HD).transpose(0, 2, 1, 3)
        q0h = q0.reshape(B, 1, H, HD).transpose(0, 2, 1, 3)      # [B,H,1,hd]

        d0 = jnp.einsum('bhid,bhjd->bhij', q0h, k) * SCALE       # [B,H,1,N]
        dm = mj[:, None, 0:1, None] & mj[:, None, None, :]
        d0 = jnp.where(dm, d0, -jnp.finfo(d0.dtype).max)
        a0 = jax.nn.softmax(d0, axis=-1)

        cls_attn = a0[:, :, 0, 1:]
        value_norms = jnp.linalg.norm(v[:, :, 1:, :], axis=-1)
        sig = jnp.sum(cls_attn * value_norms, axis=1)
        normed = sig / (jnp.sum(sig, axis=-1, keepdims=True) + EPS)
        cdf = jnp.cumsum(normed, axis=1)
        cdf = jnp.where(mj[:, 1:], cdf, cdf + 0.1)

        steps = (2.0 * jnp.arange(S, dtype=cdf.dtype) + 1.0) / (2.0 * S)
        dist = jnp.abs(steps[None, :, None] - cdf[:, None, :])
        ids = jnp.argmin(dist, axis=-1).astype(jnp.int32) + 1

        # _unique_pad, verbatim from the reference
        s = jnp.sort(ids, axis=-1)
        dup = jnp.concatenate(
            [jnp.zeros_like(s[:, :1], dtype=bool), s[:, 1:] == s[:, :-1]], axis=-1)
        u = jnp.where(dup, N, s)
        u = jnp.sort(u, axis=-1)
        uniq_core = jnp.where(u == N, 0, u)

        new_mask = jnp.concatenate(
            [jnp.ones((B, 1), dtype=bool), uniq_core != 0], axis=-1)
        uniq = jnp.concatenate(
            [jnp.zeros((B, 1), dtype=jnp.int32), uniq_core], axis=-1)
        return np.asarray(new_mask), np.asarray(uniq)


# ----------------------------------------------------------------------------
# Device kernel
# ----------------------------------------------------------------------------

def legalize_waits(nc):
    """The local walrus build only supports ONE sync wait per instruction
    (setupSyncWait raises "Too many sync wait commands" otherwise, even for
    the Tile framework's own kernel-tail Drain).  Split every multi-wait
    instruction by hoisting all but the last wait onto same-engine NoOps
    placed immediately before it — engines execute in order, so semantics
    are preserved."""
    from concourse import mybir

    ctr = 0
    for f in nc.m.functions:
        for blk in f.blocks:
            out, changed = [], False
            for inst in blk.instructions:
                si = inst.sync_info
                if si is not None and si.on_wait and len(si.on_wait) > 1:
                    waits = list(si.on_wait)
                    for w in waits[:-1]:
                        ctr += 1
                        nop = mybir.InstNoOp(name=f"WSPLIT-{ctr}", ins=[], outs=[])
                        nop.engine = inst.engine
                        nop.sync_info = mybir.SyncInfo(on_wait=[w], on_update=[])
                        nc.register_instruction(nop, overwrite=True)
                        out.append(nop)
                    si.on_wait = [waits[-1]]
                    changed = True
                out.append(inst)
            if changed:
                blk.instructions = out
    return ctr


def build_nc(bl=BL):
    import concourse.bass as bass
    import concourse.tile as tile
    from concourse import mybir

    f32 = mybir.dt.float32
    Exp = mybir.ActivationFunctionType.Exp
    fr = mybir.dt.float32r if USE_F32R else f32
    r = lambda ap: ap

    nc = bass.Bass("TRN2", target_bir_lowering=False, debug=False)

    xT_d = nc.dram_tensor("xT", [bl, C, NJ], fr, kind="ExternalInput").ap()
    xsT_d = nc.dram_tensor("xselT", [bl, C, SP2], fr, kind="ExternalInput").ap()
    qkvT_d = nc.dram_tensor("qkvT", [C, 3 * C], fr, kind="ExternalInput").ap()
    projT_d = nc.dram_tensor("projT", [C, C], fr, kind="ExternalInput").ap()
    pb_d = nc.dram_tensor("pbias", [1, C], fr, kind="ExternalInput").ap()
    em_d = nc.dram_tensor("emat", [2, 128], fr, kind="ExternalInput").ap()
    vones_d = nc.dram_tensor("vones", [128, H], fr, kind="ExternalInput").ap()
    ones_d = nc.dram_tensor("onesrow", [1, SP2], fr, kind="ExternalInput").ap()
    out_d = nc.dram_tensor("outT", [bl, C, SP2], f32, kind="ExternalOutput").ap()

    with tile.TileContext(nc) as tc:
        with (
            nc.allow_low_precision(reason="fp32r tiles share fp32 bit layout"),
            tc.tile_pool(name="wpool", bufs=1) as wp,
            tc.tile_pool(name="xpool", bufs=1) as xp,
            tc.tile_pool(name="kpool", bufs=1) as kp,
            tc.tile_pool(name="spool", bufs=1) as sp,
            tc.tile_pool(name="epool", bufs=1) as ep,
            tc.tile_pool(name="opool", bufs=1) as op,
            tc.tile_pool(name="ppb", bufs=2, space="PSUM") as ppb,
            tc.tile_pool(name="ppm", bufs=2, space="PSUM") as ppm,
            tc.tile_pool(name="ppd", bufs=3, space="PSUM") as ppd,
            tc.tile_pool(name="ppo", bufs=1, space="PSUM") as ppo,
        ):
            # ---- persistent weights/constants -------------------------------
            qkvT = [wp.tile([128, 3 * C], fr, tag=f"qkvT{t}", name=f"qkvT{t}") for t in range(CT)]
            projT = [wp.tile([128, C], fr, tag=f"projT{t}", name=f"projT{t}") for t in range(CT)]
            pb = wp.tile([1, C], fr, tag="pb", name="pb")
            em0 = wp.tile([1, 128], fr, tag="em0", name="em0")
            em1 = wp.tile([1, 128], fr, tag="em1", name="em1")
            ones = wp.tile([1, SP2], fr, tag="ones", name="ones")
            for t in range(CT):
                nc.sync.dma_start(qkvT[t][:], qkvT_d[t * 128:(t + 1) * 128, :])
                nc.sync.dma_start(projT[t][:], projT_d[t * 128:(t + 1) * 128, :])
            nc.sync.dma_start(pb[:], pb_d[:])
            nc.sync.dma_start(em0[:], em_d[0:1, :])
            nc.sync.dma_start(em1[:], em_d[1:2, :])
            nc.sync.dma_start(ones[:], ones_d[:])

            # V tiles are persistent: the per-head ones column (col 64 of each
            # 65-wide head block) is set once and never overwritten.
            vsb = [kp.tile([128, H * 65], fr, tag=f"v{jt}", name=f"v{jt}") for jt in range(5)]
            for jt in range(5):
                nc.sync.dma_start(vsb[jt][:, 64::65], vones_d[:])

            for s in range(bl):
                # ---- load inputs (double-buffered across samples) ----------
                xT = [xp.tile([128, NJ], fr, tag=f"xT{t}", bufs=2, name=f"xT{t}_{s}")
                      for t in range(CT)]
                xsT = [xp.tile([128, SP2], fr, tag=f"xsT{t}", bufs=2, name=f"xsT{t}_{s}")
                       for t in range(CT)]
                for t in range(CT):
                    nc.sync.dma_start(xT[t][:], xT_d[s, t * 128:(t + 1) * 128, :])
                    nc.sync.dma_start(xsT[t][:], xsT_d[s, t * 128:(t + 1) * 128, :])

                # ---- K_T[c',j] = Wk^T.T @ X^T ------------------------------
                kT = [kp.tile([128, NJ], fr, tag=f"k{t}", name=f"k{t}_{s}") for t in range(CT)]
                for cp in range(CT):
                    for j0, jw in ((0, 290), (290, 288)):
                        ps = ppb.tile([128, jw], f32, tag="big", name="psb")
                        for ct in range(CT):
                            nc.tensor.matmul(
                                ps[:],
                                r(qkvT[ct][:, C + cp * 128: C + (cp + 1) * 128]),
                                r(xT[ct][:, j0:j0 + jw]),
                                start=(ct == 0), stop=(ct == CT - 1))
                        nc.vector.tensor_copy(kT[cp][:, j0:j0 + jw], ps[:])

                # ---- V[j, h*65+d] natural, head-strided --------------------
                for jt in range(5):
                    jw = JT[jt]
                    for d0_, dw, h0 in ((0, 512, 0), (512, 256, 8)):
                        ps = ppb.tile([128, dw], f32, tag="big", name="psv")
                        for ct in range(CT):
                            nc.tensor.matmul(
                                ps[:jw],
                                r(xT[ct][:, jt * 128: jt * 128 + jw]),
                                r(qkvT[ct][:, 2 * C + d0_: 2 * C + d0_ + dw]),
                                start=(ct == 0), stop=(ct == CT - 1))
                        nh = dw // HD
                        dst = vsb[jt][0:jw, h0 * 65:(h0 + nh) * 65]
                        dst = dst.rearrange("p (h e) -> p h e", e=65)[:, :, 0:HD]
                        src = ps[:jw].rearrange("p (h e) -> p h e", e=HD)
                        nc.vector.tensor_copy(dst, src)

                # ---- q_sel_T[c',i] = Wq^T.T @ xsel^T -----------------------
                qsT = [sp.tile([128, SP2], fr, tag=f"q{t}", name=f"q{t}_{s}") for t in range(CT)]
                for cp in range(CT):
                    ps = ppm.tile([128, SP2], f32, tag="mid", name="psm")
                    for ct in range(CT):
                        nc.tensor.matmul(
                            ps[:], r(qkvT[ct][:, cp * 128:(cp + 1) * 128]),
                            r(xsT[ct][:]), start=(ct == 0), stop=(ct == CT - 1))
                    nc.vector.tensor_copy(qsT[cp][:], ps[:])

                # ---- per-head attention ------------------------------------
                osb = [op.tile([128, SP2], fr, tag=f"o{t}", name=f"o{t}_{s}") for t in range(CT)]
                den = op.tile([1, H * SP2], fr, tag="den", name=f"den_{s}")
                for h in range(H):
                    kt, qt, poff = kT[h // 2], qsT[h // 2], (h % 2) * 64
                    ex = []
                    for jt in range(5):
                        jw = JT[jt]
                        ps = ppd.tile([128, SP2], f32, tag="dots", name="psd")
                        nc.tensor.matmul(
                            ps[:jw],
                            r(kt[poff:poff + 64, jt * 128: jt * 128 + jw]),
                            r(qt[poff:poff + 64, :]),
                            start=True, stop=True)
                        et = ep.tile([128, SP2], fr, tag="exp", bufs=10, name="et")
                        nc.scalar.activation(et[:jw], ps[:jw], Exp, scale=SCALE)
                        ex.append(et)
                    po = ppo.tile([65, SP2], f32, tag="oaug", name="po")
                    for jt in range(5):
                        jw = JT[jt]
                        nc.tensor.matmul(
                            po[:], r(vsb[jt][0:jw, h * 65:(h + 1) * 65]),
                            r(ex[jt][:jw]), start=(jt == 0), stop=(jt == 4))
                    t, ooff = h // 2, (h % 2) * 64
                    nc.vector.tensor_copy(osb[t][ooff:ooff + 64, :], po[0:64, :])
                    nc.vector.tensor_copy(
                        den[0:1, h * SP2:(h + 1) * SP2], po[64:65, :])

                # ---- normalize: o *= 1/denom (per head, bcast via matmul) --
                rden = op.tile([1, H * SP2], fr, tag="rden", name=f"rden_{s}")
                nc.vector.reciprocal(rden[:], den[:])
                for t in range(CT):
                    ps = ppm.tile([128, SP2], f32, tag="mid", name="psm")
                    nc.tensor.matmul(
                        ps[:], r(em0[:]), r(rden[0:1, 2 * t * SP2:(2 * t + 1) * SP2]),
                        start=True, stop=False)
                    nc.tensor.matmul(
                        ps[:], r(em1[:]), r(rden[0:1, (2 * t + 1) * SP2:(2 * t + 2) * SP2]),
                        start=False, stop=True)
                    nc.vector.tensor_mul(osb[t][:], osb[t][:], ps[:])

                # ---- projection + bias, store ------------------------------
                for ep_ in range(CT):
                    ps = ppm.tile([128, SP2], f32, tag="mid", name="psm")
                    for ct in range(CT):
                        nc.tensor.matmul(
                            ps[:], r(projT[ct][:, ep_ * 128:(ep_ + 1) * 128]),
                            r(osb[ct][:]), start=(ct == 0), stop=False)
                    nc.tensor.matmul(
                        ps[:], r(pb[:, ep_ * 128:(ep_ + 1) * 128]), r(ones[:]),
                        start=False, stop=True)
                    ot = sp.tile([128, SP2], f32, tag="out", bufs=3, name="ot")
                    nc.vector.tensor_copy(ot[:], ps[:])
                    nc.sync.dma_start(
                        out_d[s, ep_ * 128:(ep_ + 1) * 128, :], ot[:])

    legalize_waits(nc)
    return nc


def make_in_maps(x, uniq, qkv_w, proj_w, proj_b, bl=BL, ncores=NCORES):
    x = np.asarray(x, dtype=np.float32)
    xsel = x[np.arange(B)[:, None], np.asarray(uniq)]          # [B,257,768]
    xT = np.zeros((B, C, NJ), np.float32)                       # zero-pad j=577
    xT[:, :, :N] = x.transpose(0, 2, 1)
    xsT = np.zeros((B, C, SP2), np.float32)                     # zero-pad i=257
    xsT[:, :, :SP1] = xsel.transpose(0, 2, 1)
    qkvT = np.ascontiguousarray(np.asarray(qkv_w, np.float32).T)
    projT = np.ascontiguousarray(np.asarray(proj_w, np.float32).T)
    pb = np.asarray(proj_b, np.float32).reshape(1, C)
    em = np.zeros((2, 128), np.float32)
    em[0, :64] = 1.0
    em[1, 64:] = 1.0
    vones = np.ones((128, H), np.float32)
    onesrow = np.ones((1, SP2), np.float32)
    maps = []
    for core in range(ncores):
        sl = slice(core * bl, (core + 1) * bl)
        maps.append({
            "xT": np.ascontiguousarray(xT[sl]),
            "xselT": np.ascontiguousarray(xsT[sl]),
            "qkvT": qkvT, "projT": projT, "pbias": pb, "emat": em,
            "vones": vones, "onesrow": onesrow,
        })
    return maps


_NC_CACHE = {}


def _get_nc():
    if "nc" not in _NC_CACHE:
        _NC_CACHE["nc"] = build_nc(BL)
    return _NC_CACHE["nc"]


def kernel(x, mask, qkv_w, proj_w, proj_b):
    from concourse.bass_utils import run_bass_kernel_spmd

    new_mask, uniq = _host_selection(x, mask, qkv_w)
    in_maps = make_in_maps(x, uniq, qkv_w, proj_w, proj_b)
    nc = _get_nc()
    res = run_bass_kernel_spmd(nc, in_maps, list(range(NCORES)))
    out = np.empty((B, SP1, C), np.float32)
    for core in range(NCORES):
        oT = res.results[core]["outT"]                          # [BL,768,258]
        for lb in range(BL):
            out[core * BL + lb] = oT[lb, :, :SP1].T
    return out, new_mask, uniq
